# revision 3
# baseline (speedup 1.0000x reference)
"""Causal self-attention (B=4, T=2048, C=1024, 16 heads) on 8 Trainium2 cores.

Sharding: core c -> batch b = c//2 (4 data-parallel groups), head shard
s = c%2 (Megatron tensor-parallel: 8 of 16 heads, qkv column-sharded,
proj row-sharded).  Each core computes a partial projection output for
its batch; the host sums the two partials per batch (+ b_proj).

v2 restructure vs v1: the P~@V matmul is flipped to q-major
(lhsT = P~^T block [k,q], rhs = V-hat [k, 65]) so each instruction
streams only 65 columns instead of 512 (PE cost on TRN2 is purely the
moving-operand free size).  The softmax denominator Z lands as a PSUM
*column* (ones-column of V-hat), so normalization is a per-partition
reciprocal + tensor_scalar multiply, replacing the v1
reciprocal/partition-broadcast/DMA-shift chain.  The normalized tile is
PE-transposed back to feature-major for the row-sharded projection.
"""

import numpy as np
import ml_dtypes
from contextlib import ExitStack

import concourse.bass as bass
import concourse.tile as tile
from concourse import mybir, bacc
from concourse.bass_utils import run_bass_kernel_spmd
from concourse.masks import make_identity

F32 = mybir.dt.float32
BF16 = mybir.dt.bfloat16
AF = mybir.ActivationFunctionType
ALU = mybir.AluOpType

B, T, C = 4, 2048, 1024
NH, DH = 16, 64
SCALE = 1.0 / float(np.sqrt(DH))
NCORES = 8
HPC = 8              # heads per core
WCOLS = HPC * DH     # 512 qkv columns per core
NPAIR = HPC // 2     # head pairs (row/psum packing unit)
KC = T // 128        # 16 key-token chunks
QC = T // 512        # 4 query chunks
FC = C // 128        # 8 feature chunks


def _build_program(use_bias: bool):
    nc = bacc.Bacc(trn_type="TRN2", target_bir_lowering=False, debug=False)

    xT = nc.dram_tensor("xT", [C, T], BF16, kind="ExternalInput").ap()
    wq = nc.dram_tensor("wq", [C, WCOLS], BF16, kind="ExternalInput").ap()
    wk = nc.dram_tensor("wk", [C, WCOLS], BF16, kind="ExternalInput").ap()
    wv = nc.dram_tensor("wv", [C, WCOLS], BF16, kind="ExternalInput").ap()
    wp = nc.dram_tensor("wp", [WCOLS, C], BF16, kind="ExternalInput").ap()
    if use_bias:
        bq = nc.dram_tensor("bq", [WCOLS], F32, kind="ExternalInput").ap()
        bk = nc.dram_tensor("bk", [WCOLS], F32, kind="ExternalInput").ap()
        bv = nc.dram_tensor("bv", [WCOLS], F32, kind="ExternalInput").ap()
    out = nc.dram_tensor("out", [T, C], F32, kind="ExternalOutput").ap()

    with tile.TileContext(nc) as tc, ExitStack() as ctx:
        pool = ctx.enter_context(tc.tile_pool(name="main", bufs=1))
        xpool = ctx.enter_context(tc.tile_pool(name="xt", bufs=3))
        ptpool = ctx.enter_context(tc.tile_pool(name="pt", bufs=36))
        ynpool = ctx.enter_context(tc.tile_pool(name="yn", bufs=4))
        zpool = ctx.enter_context(tc.tile_pool(name="zr", bufs=4))
        opool = ctx.enter_context(tc.tile_pool(name="out", bufs=3))
        ps_mm = ctx.enter_context(tc.tile_pool(name="ps_mm", bufs=2, space="PSUM"))
        ps_s = ctx.enter_context(tc.tile_pool(name="ps_s", bufs=2, space="PSUM"))
        ps_y = ctx.enter_context(tc.tile_pool(name="ps_y", bufs=2, space="PSUM"))

        QT = [pool.tile([128, T], BF16, tag=f"qt{p}", name=f"qt{p}") for p in range(NPAIR)]
        KT = [pool.tile([128, T], BF16, tag=f"kt{p}", name=f"kt{p}") for p in range(NPAIR)]
        # V tiles head-major with a trailing ones column per head: [tok, h, 65]
        V = [pool.tile([128, HPC, DH + 1], BF16, tag=f"v{t}", name=f"v{t}") for t in range(KC)]
        for t in range(KC):
            nc.vector.memset(V[t][:, :, DH : DH + 1], 1.0)
        YT = [pool.tile([128, T], BF16, tag=f"yt{p}", name=f"yt{p}") for p in range(NPAIR)]

        # PE p-state warm-up: ~4us of dependency-free zero matmuls so the
        # tensor engine reaches full clock before the first real operands
        # arrive from HBM (ramp needs ~3us of continuous busy).
        zdummy = pool.tile([128, 512], BF16, tag="zdummy", name="zdummy")
        nc.vector.memset(zdummy, 0.0)
        for wi in range(8):
            wps = ps_y.tile([128, 512], F32, tag="y", name="warm")
            nc.tensor.matmul(
                wps, lhsT=zdummy[:, 0:128], rhs=zdummy, start=True, stop=True
            )

        wq_t = pool.tile([128, FC, WCOLS], BF16, tag="wq", name="wq_t")
        wk_t = pool.tile([128, FC, WCOLS], BF16, tag="wk", name="wk_t")
        wv_t = pool.tile([128, FC, WCOLS], BF16, tag="wv", name="wv_t")
        wp_t = pool.tile([128, NPAIR, C], BF16, tag="wp", name="wp_t")
        wq_sb = [wq_t[:, f, :] for f in range(FC)]
        wk_sb = [wk_t[:, f, :] for f in range(FC)]
        wv_sb = [wv_t[:, f, :] for f in range(FC)]
        wp_sb = [wp_t[:, p, :] for p in range(NPAIR)]

        def issue_xt(t4):
            tok = slice(t4 * 512, (t4 + 1) * 512)
            xt_t = xpool.tile([128, FC, 512], BF16, tag="x", name="x")
            nc.sync.dma_start(
                xt_t, xT.rearrange("(f p) t -> p f t", p=128)[:, :, tok]
            )
            return [xt_t[:, f, :] for f in range(FC)]

        # DMA issue order sets queue priority: slab-0 activations and the
        # first-needed weights land first, wp (only needed by proj) last.
        xt0 = issue_xt(0)
        nc.sync.dma_start(wv_t, wv.rearrange("(f p) n -> p f n", p=128))
        nc.sync.dma_start(wq_t, wq.rearrange("(f p) n -> p f n", p=128))
        nc.sync.dma_start(wk_t, wk.rearrange("(f p) n -> p f n", p=128))
        nc.sync.dma_start(wp_t, wp.rearrange("(g p) n -> p g n", p=128))

        if use_bias:
            bq_sb = pool.tile([128, NPAIR], F32)
            bk_sb = pool.tile([128, NPAIR], F32)
            nc.sync.dma_start(bq_sb, bq.rearrange("(c p) -> p c", p=128))
            nc.sync.dma_start(bk_sb, bk.rearrange("(c p) -> p c", p=128))
            bv_sb = pool.tile([128, WCOLS], F32)
            bv_bcast = bass.AP(
                tensor=bv.tensor, offset=bv.offset, ap=[[0, 128], *bv.ap]
            )
            nc.sync.dma_start(bv_sb, bv_bcast)

        # ====== fully interleaved pipeline over 512-token slabs ======

        def emit_qkv_slab(t4, xt=None):
            tok = slice(t4 * 512, (t4 + 1) * 512)
            if xt is None:
                xt = issue_xt(t4)

            # Emission order inside a slab: pair-p Q^T/K^T first so
            # attention for pair p unblocks after 2 psum groups, V chunks
            # spread between (needed only by the trailing Y phase).
            def emit_v(tt):
                kci = t4 * 4 + tt
                ps = ps_mm.tile([128, 512], F32, tag="ps", name="ps")
                for f in range(FC):
                    nc.tensor.matmul(
                        ps,
                        lhsT=xt[f][:, tt * 128 : (tt + 1) * 128],
                        rhs=wv_sb[f],
                        start=(f == 0),
                        stop=(f == FC - 1),
                    )
                psv = ps.rearrange("p (h d) -> p h d", h=HPC)
                if use_bias:
                    nc.vector.tensor_add(
                        V[kci][:, :, 0:DH],
                        psv,
                        bv_sb.rearrange("p (h d) -> p h d", h=HPC),
                    )
                else:
                    nc.vector.tensor_copy(V[kci][:, :, 0:DH], psv)

            def emit_qk(wsb, dst, bias, p):
                ps = ps_mm.tile([128, 512], F32, tag="ps", name="ps")
                for f in range(FC):
                    nc.tensor.matmul(
                        ps,
                        lhsT=wsb[f][:, p * 128 : (p + 1) * 128],
                        rhs=xt[f],
                        start=(f == 0),
                        stop=(f == FC - 1),
                    )
                if use_bias:
                    bsb = bq_sb if bias == "bq" else bk_sb
                    nc.scalar.activation(
                        dst[p][:, tok], ps, AF.Copy, bias=bsb[:, p : p + 1]
                    )
                else:
                    nc.vector.tensor_copy(dst[p][:, tok], ps)

            for tt in range(4):
                emit_v(tt)
            for p in range(NPAIR):
                emit_qk(wq_sb, QT, "bq", p)
                emit_qk(wk_sb, KT, "bk", p)

        PTS = {}

        def emit_attention_s(p, q):
            nblk = 4 * q + 4
            # --- S + exp phase: P~^T tiles [k, h, q] for all k blocks ---
            pts = []
            for k in range(nblk):
                # diagonal offset: columns q < d of this block are
                # fully masked -> restrict all work to q >= d
                d = max(0, 128 * k - 512 * q)
                # S^T block [128 k, 512-d q], both heads row-tiled
                s = ps_s.tile([128, 1024], F32, tag="s", name="s")
                pt = ptpool.tile([128, 2, 512], BF16, tag="pt", name="pt")
                for h in (0, 1):
                    nc.tensor.matmul(
                        s[:, h * 512 + d : (h + 1) * 512],
                        lhsT=KT[p][h * 64 : (h + 1) * 64, k * 128 : (k + 1) * 128],
                        rhs=QT[p][h * 64 : (h + 1) * 64, q * 512 + d : (q + 1) * 512],
                        start=True,
                        stop=True,
                    )
                sv = s.rearrange("p (h q) -> p h q", h=2)
                nc.scalar.activation(
                    pt[:, :, d:512], sv[:, :, d:512], AF.Exp, scale=SCALE
                )
                if k >= 4 * q:
                    # triangular boundary band: zero where q_b < k
                    nc.gpsimd.affine_select(
                        out=pt[:, :, d : d + 128],
                        in_=pt[:, :, d : d + 128],
                        compare_op=ALU.is_ge,
                        fill=0.0,
                        base=0,
                        channel_multiplier=-1,
                        pattern=[[0, 2], [1, 128]],
                    )
                pts.append(pt)
            PTS[(p, q)] = pts

        def emit_attention_y(p, q):
            pts = PTS.pop((p, q))
            # --- Y phase: q-major accumulation, one q-subtile at a time.
            # The two heads' accumulation groups MUST live in different PSUM
            # banks (interleaved groups in one bank corrupt each other), so
            # each head gets its own ring slot.  The feature-major transpose
            # goes through the DMA xbar, not the PE/PSUM. ---
            for qq in range(4):
                qsub = 4 * q + qq  # global 128-token row block
                yh = [ps_y.tile([128, 65], F32, tag="y", name=f"yh{h}") for h in (0, 1)]
                for k in range(qsub + 1):
                    for h in (0, 1):
                        nc.tensor.matmul(
                            yh[h],
                            lhsT=pts[k][:, h, qq * 128 : (qq + 1) * 128],
                            rhs=V[k][:, 2 * p + h, :],
                            start=(k == 0),
                            stop=(k == qsub),
                        )
                zi = zpool.tile([128, 2], F32, tag="zi", name="zi")
                yn = ynpool.tile([128, 128], BF16, tag="yn", name="yn")
                for h in (0, 1):
                    nc.vector.reciprocal(zi[:, h : h + 1], yh[h][:, 64:65])
                    nc.vector.tensor_scalar_mul(
                        yn[:, h * 64 : (h + 1) * 64], yh[h][:, 0:64], zi[:, h : h + 1]
                    )
                nc.sync.dma_start_transpose(
                    YT[p][:, qsub * 128 : (qsub + 1) * 128], yn
                )

        def emit_proj(tt):
            # proj psum lives in the qkv ring but is emitted after all qkv,
            # so its slot-reuse never gates qkv; consumers are un-gated
            # copies.  One merged [128,1024] DMA per token block (each DMA
            # costs a fixed slot on the single hardware DGE queue).
            o = opool.tile([128, C], F32, tag="o", name="o")
            for n2 in range(2):
                nsl = slice(n2 * 512, (n2 + 1) * 512)
                ps = ps_mm.tile([128, 512], F32, tag="ps", name="psp")
                for p in range(NPAIR):
                    nc.tensor.matmul(
                        ps,
                        lhsT=YT[p][:, tt * 128 : (tt + 1) * 128],
                        rhs=wp_sb[p][:, nsl],
                        start=(p == 0),
                        stop=(p == NPAIR - 1),
                    )
                nc.vector.tensor_copy(o[:, nsl], ps)
                if tt >= 4 * QC - 2:
                    # drain the final token blocks in halves so the out DMA
                    # overlaps the second half's copy
                    nc.sync.dma_start(out[tt * 128 : (tt + 1) * 128, nsl], o[:, nsl])
            if tt < 4 * QC - 2:
                nc.sync.dma_start(out[tt * 128 : (tt + 1) * 128, :], o)

        # Priority order (emission = scheduler priority): attention(q) above
        # qkv(q+1), with all proj at the lowest priority so its PE work acts
        # as stall filler during the exp-bound late attention windows.
        # Priority shape: S/exp feeds the Activation engine (the long serial
        # chain) as early as possible; each pair's Y phase trails one pair
        # behind its S phase (act-free PE work = stall filler), qkv(q+1) and
        # proj(q) below the chunk's attention.
        emit_qkv_slab(0, xt0)
        for q in range(QC):
            emit_attention_s(0, q)
            for p in range(1, NPAIR):
                emit_attention_s(p, q)
                emit_attention_y(p - 1, q)
            emit_attention_y(NPAIR - 1, q)
            if q + 1 < QC:
                emit_qkv_slab(q + 1)
        for tt in range(4 * QC):
            emit_proj(tt)

    nc.compile()
    return nc


_PROGRAMS: dict = {}


def _get_program(use_bias: bool):
    if use_bias not in _PROGRAMS:
        _PROGRAMS[use_bias] = _build_program(use_bias)
    return _PROGRAMS[use_bias]


def _bf16(a):
    return np.ascontiguousarray(a.astype(ml_dtypes.bfloat16))


def kernel(x, W_qkv, b_qkv, W_proj, b_proj):
    x = np.asarray(x, dtype=np.float32)
    W_qkv = np.asarray(W_qkv, dtype=np.float32)
    b_qkv = np.asarray(b_qkv, dtype=np.float32)
    W_proj = np.asarray(W_proj, dtype=np.float32)
    b_proj = np.asarray(b_proj, dtype=np.float32)

    use_bias = bool(np.any(b_qkv != 0.0))
    nc = _get_program(use_bias)

    xTb = np.ascontiguousarray(x.transpose(0, 2, 1))  # [B, C, T] f32

    in_maps = []
    for c in range(NCORES):
        b, s = c // 2, c % 2
        m = {
            "xT": _bf16(xTb[b]),
            "wq": _bf16(W_qkv[:, s * WCOLS : (s + 1) * WCOLS]),
            "wk": _bf16(W_qkv[:, C + s * WCOLS : C + (s + 1) * WCOLS]),
            "wv": _bf16(W_qkv[:, 2 * C + s * WCOLS : 2 * C + (s + 1) * WCOLS]),
            "wp": _bf16(W_proj[s * WCOLS : (s + 1) * WCOLS, :]),
        }
        if use_bias:
            m["bq"] = np.ascontiguousarray(b_qkv[s * WCOLS : (s + 1) * WCOLS])
            m["bk"] = np.ascontiguousarray(b_qkv[C + s * WCOLS : C + (s + 1) * WCOLS])
            m["bv"] = np.ascontiguousarray(
                b_qkv[2 * C + s * WCOLS : 2 * C + (s + 1) * WCOLS]
            )
        in_maps.append(m)

    res = run_bass_kernel_spmd(nc, in_maps, list(range(NCORES))).results

    outp = np.empty((B, T, C), dtype=np.float32)
    for b in range(B):
        outp[b] = res[2 * b]["out"] + res[2 * b + 1]["out"]
    outp += b_proj
    return outp


def modeled_ns(use_bias: bool = False) -> float:
    """Single-core cost-model estimate of the kernel duration."""
    from concourse.timeline_sim import TimelineSim

    return TimelineSim(_build_program(use_bias)).simulate()


# revision 4
# speedup vs baseline: 1.0096x; 1.0096x over previous
"""Causal self-attention (B=4, T=2048, C=1024, 16 heads) on 8 Trainium2 cores.

Sharding: core c -> batch b = c//2 (4 data-parallel groups), head shard
s = c%2 (Megatron tensor-parallel: 8 of 16 heads, qkv column-sharded,
proj row-sharded).  Each core computes a partial projection output for
its batch; the host sums the two partials per batch (+ b_proj).

v2 restructure vs v1: the P~@V matmul is flipped to q-major
(lhsT = P~^T block [k,q], rhs = V-hat [k, 65]) so each instruction
streams only 65 columns instead of 512 (PE cost on TRN2 is purely the
moving-operand free size).  The softmax denominator Z lands as a PSUM
*column* (ones-column of V-hat), so normalization is a per-partition
reciprocal + tensor_scalar multiply, replacing the v1
reciprocal/partition-broadcast/DMA-shift chain.  The normalized tile is
PE-transposed back to feature-major for the row-sharded projection.
"""

import numpy as np
import ml_dtypes
from contextlib import ExitStack

import concourse.bass as bass
import concourse.tile as tile
from concourse import mybir, bacc
from concourse.bass_utils import run_bass_kernel_spmd
from concourse.masks import make_identity

F32 = mybir.dt.float32
BF16 = mybir.dt.bfloat16
AF = mybir.ActivationFunctionType
ALU = mybir.AluOpType

B, T, C = 4, 2048, 1024
NH, DH = 16, 64
SCALE = 1.0 / float(np.sqrt(DH))
NCORES = 8
HPC = 8              # heads per core
WCOLS = HPC * DH     # 512 qkv columns per core
NPAIR = HPC // 2     # head pairs (row/psum packing unit)
KC = T // 128        # 16 key-token chunks
QC = T // 512        # 4 query chunks
FC = C // 128        # 8 feature chunks


def _build_program(use_bias: bool):
    nc = bacc.Bacc(trn_type="TRN2", target_bir_lowering=False, debug=False)

    xT = nc.dram_tensor("xT", [C, T], BF16, kind="ExternalInput").ap()
    wq = nc.dram_tensor("wq", [C, WCOLS], BF16, kind="ExternalInput").ap()
    wk = nc.dram_tensor("wk", [C, WCOLS], BF16, kind="ExternalInput").ap()
    wv = nc.dram_tensor("wv", [C, WCOLS], BF16, kind="ExternalInput").ap()
    wp = nc.dram_tensor("wp", [WCOLS, C], BF16, kind="ExternalInput").ap()
    if use_bias:
        bq = nc.dram_tensor("bq", [WCOLS], F32, kind="ExternalInput").ap()
        bk = nc.dram_tensor("bk", [WCOLS], F32, kind="ExternalInput").ap()
        bv = nc.dram_tensor("bv", [WCOLS], F32, kind="ExternalInput").ap()
    out = nc.dram_tensor("out", [T, C], F32, kind="ExternalOutput").ap()

    with tile.TileContext(nc) as tc, ExitStack() as ctx:
        pool = ctx.enter_context(tc.tile_pool(name="main", bufs=1))
        xpool = ctx.enter_context(tc.tile_pool(name="xt", bufs=3))
        ptpool = ctx.enter_context(tc.tile_pool(name="pt", bufs=36))
        ynpool = ctx.enter_context(tc.tile_pool(name="yn", bufs=4))
        zpool = ctx.enter_context(tc.tile_pool(name="zr", bufs=4))
        opool = ctx.enter_context(tc.tile_pool(name="out", bufs=3))
        ps_mm = ctx.enter_context(tc.tile_pool(name="ps_mm", bufs=2, space="PSUM"))
        ps_s = ctx.enter_context(tc.tile_pool(name="ps_s", bufs=2, space="PSUM"))
        ps_y = ctx.enter_context(tc.tile_pool(name="ps_y", bufs=2, space="PSUM"))

        QT = [pool.tile([128, T], BF16, tag=f"qt{p}", name=f"qt{p}") for p in range(NPAIR)]
        KT = [pool.tile([128, T], BF16, tag=f"kt{p}", name=f"kt{p}") for p in range(NPAIR)]
        # V tiles head-major with a trailing ones column per head: [tok, h, 65]
        V = [pool.tile([128, HPC, DH + 1], BF16, tag=f"v{t}", name=f"v{t}") for t in range(KC)]
        for t in range(KC):
            nc.vector.memset(V[t][:, :, DH : DH + 1], 1.0)
        YT = [pool.tile([128, T], BF16, tag=f"yt{p}", name=f"yt{p}") for p in range(NPAIR)]

        # PE p-state warm-up: ~4us of dependency-free zero matmuls so the
        # tensor engine reaches full clock before the first real operands
        # arrive from HBM (ramp needs ~3us of continuous busy).
        zdummy = pool.tile([128, 512], BF16, tag="zdummy", name="zdummy")
        nc.vector.memset(zdummy, 0.0)
        for wi in range(8):
            wps = ps_y.tile([128, 512], F32, tag="y", name="warm")
            nc.tensor.matmul(
                wps, lhsT=zdummy[:, 0:128], rhs=zdummy, start=True, stop=True
            )

        wq_t = pool.tile([128, FC, WCOLS], BF16, tag="wq", name="wq_t")
        wk_t = pool.tile([128, FC, WCOLS], BF16, tag="wk", name="wk_t")
        wv_t = pool.tile([128, FC, WCOLS], BF16, tag="wv", name="wv_t")
        wp_t = pool.tile([128, NPAIR, C], BF16, tag="wp", name="wp_t")
        wq_sb = [wq_t[:, f, :] for f in range(FC)]
        wk_sb = [wk_t[:, f, :] for f in range(FC)]
        wv_sb = [wv_t[:, f, :] for f in range(FC)]
        wp_sb = [wp_t[:, p, :] for p in range(NPAIR)]

        def issue_xt(t4):
            tok = slice(t4 * 512, (t4 + 1) * 512)
            xt_t = xpool.tile([128, FC, 512], BF16, tag="x", name="x")
            nc.sync.dma_start(
                xt_t, xT.rearrange("(f p) t -> p f t", p=128)[:, :, tok]
            )
            return [xt_t[:, f, :] for f in range(FC)]

        # DMA issue order sets queue priority: slab-0 activations and the
        # first-needed weights land first, wp (only needed by proj) last.
        xt0_t = xpool.tile([128, FC, 512], BF16, tag="x", name="x")
        xr = xT.rearrange("(f p) t -> p f t", p=128)
        wvr = wv.rearrange("(f p) n -> p f n", p=128)
        for jj in range(4):
            nc.sync.dma_start(xt0_t[:, 2*jj:2*jj+2, :], xr[:, 2*jj:2*jj+2, 0:512])
            nc.sync.dma_start(wv_t[:, 2*jj:2*jj+2, :], wvr[:, 2*jj:2*jj+2, :])
        xt0 = [xt0_t[:, f, :] for f in range(FC)]
        wqr = wq.rearrange("(f p) n -> p f n", p=128)
        nc.sync.dma_start(wq_t[:, 0:4, :], wqr[:, 0:4, :])
        nc.sync.dma_start(wq_t[:, 4:8, :], wqr[:, 4:8, :])
        nc.sync.dma_start(wk_t, wk.rearrange("(f p) n -> p f n", p=128))
        nc.sync.dma_start(wp_t, wp.rearrange("(g p) n -> p g n", p=128))

        if use_bias:
            bq_sb = pool.tile([128, NPAIR], F32)
            bk_sb = pool.tile([128, NPAIR], F32)
            nc.sync.dma_start(bq_sb, bq.rearrange("(c p) -> p c", p=128))
            nc.sync.dma_start(bk_sb, bk.rearrange("(c p) -> p c", p=128))
            bv_sb = pool.tile([128, WCOLS], F32)
            bv_bcast = bass.AP(
                tensor=bv.tensor, offset=bv.offset, ap=[[0, 128], *bv.ap]
            )
            nc.sync.dma_start(bv_sb, bv_bcast)

        # ====== fully interleaved pipeline over 512-token slabs ======

        def emit_qkv_slab(t4, xt=None):
            tok = slice(t4 * 512, (t4 + 1) * 512)
            if xt is None:
                xt = issue_xt(t4)

            # Emission order inside a slab: pair-p Q^T/K^T first so
            # attention for pair p unblocks after 2 psum groups, V chunks
            # spread between (needed only by the trailing Y phase).
            def emit_v(tt):
                kci = t4 * 4 + tt
                ps = ps_mm.tile([128, 512], F32, tag="ps", name="ps")
                for f in range(FC):
                    nc.tensor.matmul(
                        ps,
                        lhsT=xt[f][:, tt * 128 : (tt + 1) * 128],
                        rhs=wv_sb[f],
                        start=(f == 0),
                        stop=(f == FC - 1),
                    )
                psv = ps.rearrange("p (h d) -> p h d", h=HPC)
                if use_bias:
                    nc.vector.tensor_add(
                        V[kci][:, :, 0:DH],
                        psv,
                        bv_sb.rearrange("p (h d) -> p h d", h=HPC),
                    )
                else:
                    nc.vector.tensor_copy(V[kci][:, :, 0:DH], psv)

            def emit_qk(wsb, dst, bias, p):
                ps = ps_mm.tile([128, 512], F32, tag="ps", name="ps")
                for f in range(FC):
                    nc.tensor.matmul(
                        ps,
                        lhsT=wsb[f][:, p * 128 : (p + 1) * 128],
                        rhs=xt[f],
                        start=(f == 0),
                        stop=(f == FC - 1),
                    )
                if use_bias:
                    bsb = bq_sb if bias == "bq" else bk_sb
                    nc.scalar.activation(
                        dst[p][:, tok], ps, AF.Copy, bias=bsb[:, p : p + 1]
                    )
                else:
                    nc.vector.tensor_copy(dst[p][:, tok], ps)

            for tt in range(4):
                emit_v(tt)
            for p in range(NPAIR):
                emit_qk(wq_sb, QT, "bq", p)
                emit_qk(wk_sb, KT, "bk", p)

        PTS = {}

        def emit_attention_s(p, q):
            nblk = 4 * q + 4
            # --- S + exp phase: P~^T tiles [k, h, q] for all k blocks ---
            pts = []
            for k in range(nblk):
                # diagonal offset: columns q < d of this block are
                # fully masked -> restrict all work to q >= d
                d = max(0, 128 * k - 512 * q)
                # S^T block [128 k, 512-d q], both heads row-tiled
                s = ps_s.tile([128, 1024], F32, tag="s", name="s")
                pt = ptpool.tile([128, 2, 512], BF16, tag="pt", name="pt")
                for h in (0, 1):
                    nc.tensor.matmul(
                        s[:, h * 512 + d : (h + 1) * 512],
                        lhsT=KT[p][h * 64 : (h + 1) * 64, k * 128 : (k + 1) * 128],
                        rhs=QT[p][h * 64 : (h + 1) * 64, q * 512 + d : (q + 1) * 512],
                        start=True,
                        stop=True,
                    )
                sv = s.rearrange("p (h q) -> p h q", h=2)
                nc.scalar.activation(
                    pt[:, :, d:512], sv[:, :, d:512], AF.Exp, scale=SCALE
                )
                if k >= 4 * q:
                    # triangular boundary band: zero where q_b < k
                    nc.gpsimd.affine_select(
                        out=pt[:, :, d : d + 128],
                        in_=pt[:, :, d : d + 128],
                        compare_op=ALU.is_ge,
                        fill=0.0,
                        base=0,
                        channel_multiplier=-1,
                        pattern=[[0, 2], [1, 128]],
                    )
                pts.append(pt)
            PTS[(p, q)] = pts

        def emit_attention_y(p, q):
            pts = PTS.pop((p, q))
            # --- Y phase: q-major accumulation, one q-subtile at a time.
            # The two heads' accumulation groups MUST live in different PSUM
            # banks (interleaved groups in one bank corrupt each other), so
            # each head gets its own ring slot.  The feature-major transpose
            # goes through the DMA xbar, not the PE/PSUM. ---
            for qq in range(4):
                qsub = 4 * q + qq  # global 128-token row block
                yh = [ps_y.tile([128, 65], F32, tag="y", name=f"yh{h}") for h in (0, 1)]
                for k in range(qsub + 1):
                    for h in (0, 1):
                        nc.tensor.matmul(
                            yh[h],
                            lhsT=pts[k][:, h, qq * 128 : (qq + 1) * 128],
                            rhs=V[k][:, 2 * p + h, :],
                            start=(k == 0),
                            stop=(k == qsub),
                        )
                zi = zpool.tile([128, 2], F32, tag="zi", name="zi")
                yn = ynpool.tile([128, 128], BF16, tag="yn", name="yn")
                for h in (0, 1):
                    nc.vector.reciprocal(zi[:, h : h + 1], yh[h][:, 64:65])
                    nc.vector.tensor_scalar_mul(
                        yn[:, h * 64 : (h + 1) * 64], yh[h][:, 0:64], zi[:, h : h + 1]
                    )
                nc.sync.dma_start_transpose(
                    YT[p][:, qsub * 128 : (qsub + 1) * 128], yn
                )

        def emit_proj(tt):
            # proj psum lives in the qkv ring but is emitted after all qkv,
            # so its slot-reuse never gates qkv; consumers are un-gated
            # copies.  One merged [128,1024] DMA per token block (each DMA
            # costs a fixed slot on the single hardware DGE queue).
            o = opool.tile([128, C], F32, tag="o", name="o")
            for n2 in range(2):
                nsl = slice(n2 * 512, (n2 + 1) * 512)
                ps = ps_mm.tile([128, 512], F32, tag="ps", name="psp")
                for p in range(NPAIR):
                    nc.tensor.matmul(
                        ps,
                        lhsT=YT[p][:, tt * 128 : (tt + 1) * 128],
                        rhs=wp_sb[p][:, nsl],
                        start=(p == 0),
                        stop=(p == NPAIR - 1),
                    )
                nc.vector.tensor_copy(o[:, nsl], ps)
                if tt >= 4 * QC - 2:
                    # drain the final token blocks in halves so the out DMA
                    # overlaps the second half's copy
                    nc.sync.dma_start(out[tt * 128 : (tt + 1) * 128, nsl], o[:, nsl])
            if tt < 4 * QC - 2:
                nc.sync.dma_start(out[tt * 128 : (tt + 1) * 128, :], o)

        # Priority order (emission = scheduler priority): attention(q) above
        # qkv(q+1), with all proj at the lowest priority so its PE work acts
        # as stall filler during the exp-bound late attention windows.
        # Priority shape: S/exp feeds the Activation engine (the long serial
        # chain) as early as possible; each pair's Y phase trails one pair
        # behind its S phase (act-free PE work = stall filler), qkv(q+1) and
        # proj(q) below the chunk's attention.
        emit_qkv_slab(0, xt0)
        for q in range(QC):
            emit_attention_s(0, q)
            for p in range(1, NPAIR):
                emit_attention_s(p, q)
                emit_attention_y(p - 1, q)
            emit_attention_y(NPAIR - 1, q)
            if q + 1 < QC:
                emit_qkv_slab(q + 1)
        for tt in range(4 * QC):
            emit_proj(tt)

    nc.compile()
    return nc


_PROGRAMS: dict = {}


def _get_program(use_bias: bool):
    if use_bias not in _PROGRAMS:
        _PROGRAMS[use_bias] = _build_program(use_bias)
    return _PROGRAMS[use_bias]


def _bf16(a):
    return np.ascontiguousarray(a.astype(ml_dtypes.bfloat16))


def kernel(x, W_qkv, b_qkv, W_proj, b_proj):
    x = np.asarray(x, dtype=np.float32)
    W_qkv = np.asarray(W_qkv, dtype=np.float32)
    b_qkv = np.asarray(b_qkv, dtype=np.float32)
    W_proj = np.asarray(W_proj, dtype=np.float32)
    b_proj = np.asarray(b_proj, dtype=np.float32)

    use_bias = bool(np.any(b_qkv != 0.0))
    nc = _get_program(use_bias)

    xTb = np.ascontiguousarray(x.transpose(0, 2, 1))  # [B, C, T] f32

    in_maps = []
    for c in range(NCORES):
        b, s = c // 2, c % 2
        m = {
            "xT": _bf16(xTb[b]),
            "wq": _bf16(W_qkv[:, s * WCOLS : (s + 1) * WCOLS]),
            "wk": _bf16(W_qkv[:, C + s * WCOLS : C + (s + 1) * WCOLS]),
            "wv": _bf16(W_qkv[:, 2 * C + s * WCOLS : 2 * C + (s + 1) * WCOLS]),
            "wp": _bf16(W_proj[s * WCOLS : (s + 1) * WCOLS, :]),
        }
        if use_bias:
            m["bq"] = np.ascontiguousarray(b_qkv[s * WCOLS : (s + 1) * WCOLS])
            m["bk"] = np.ascontiguousarray(b_qkv[C + s * WCOLS : C + (s + 1) * WCOLS])
            m["bv"] = np.ascontiguousarray(
                b_qkv[2 * C + s * WCOLS : 2 * C + (s + 1) * WCOLS]
            )
        in_maps.append(m)

    res = run_bass_kernel_spmd(nc, in_maps, list(range(NCORES))).results

    outp = np.empty((B, T, C), dtype=np.float32)
    for b in range(B):
        outp[b] = res[2 * b]["out"] + res[2 * b + 1]["out"]
    outp += b_proj
    return outp


def modeled_ns(use_bias: bool = False) -> float:
    """Single-core cost-model estimate of the kernel duration."""
    from concourse.timeline_sim import TimelineSim

    return TimelineSim(_build_program(use_bias)).simulate()


# revision 6
# speedup vs baseline: 1.0368x; 1.0269x over previous
"""Causal self-attention (B=4, T=2048, C=1024, 16 heads) on 8 Trainium2 cores.

Sharding: core c -> batch b = c//2 (4 data-parallel groups), head shard
s = c%2 (Megatron tensor-parallel: 8 of 16 heads, qkv column-sharded,
proj row-sharded).  Each core computes a partial projection output for
its batch; the host sums the two partials per batch (+ b_proj).

Pipeline design (evolved from a Y^T-oriented baseline, 365us -> 224us
modeled):
  * P~@V is q-major: lhsT = P~^T block [k,q-sub], rhs = V-hat [k, 65]
    so each matmul streams 65 columns instead of 512 (tensor-engine
    cost is purely the moving-operand free size).  The ones-column of
    V-hat lands the softmax denominator Z as a PSUM *column*, making
    normalization a per-partition reciprocal + tensor_scalar multiply.
  * The two heads' PV accumulation groups live in separate PSUM ring
    slots (= separate banks): interleaved accumulation groups sharing
    a bank corrupt each other.
  * The normalized [q, c] tile returns to feature-major via the DMA
    xbar (dma_start_transpose), keeping the PE and PSUM out of it.
  * Emission order = scheduler priority + pool-ring slot order: S/exp
    feeds the Activation engine (the serial softmax chain) as early as
    possible; each pair's Y phase trails one pair; qkv(q+1) sits below
    attention(q); all projection work is emitted last as pure stall
    filler.  proj shares the qkv psum ring but never precedes a qkv
    slab there, so ring reuse cannot gate qkv.
  * DMAs are merged (one per weight tensor / x-slab / out token-block,
    plus split first-arrivals for the prologue): each DMA costs a
    fixed slot on the single hardware DGE queue.
  * A short burst of zero-matmuls at t=0 ramps the PE p-state while
    the first DMAs land.
"""

import numpy as np
import ml_dtypes
from contextlib import ExitStack

import concourse.bass as bass
import concourse.tile as tile
from concourse.masks import make_identity
from concourse import mybir, bacc
from concourse.bass_utils import run_bass_kernel_spmd

F32 = mybir.dt.float32
BF16 = mybir.dt.bfloat16
AF = mybir.ActivationFunctionType
ALU = mybir.AluOpType

B, T, C = 4, 2048, 1024
NH, DH = 16, 64
SCALE = 1.0 / float(np.sqrt(DH))
NCORES = 8
HPC = 8              # heads per core
WCOLS = HPC * DH     # 512 qkv columns per core
NPAIR = HPC // 2     # head pairs (row/psum packing unit)
KC = T // 128        # 16 key-token chunks
QC = T // 512        # 4 query chunks
FC = C // 128        # 8 feature chunks


def _build_program(use_bias: bool):
    nc = bacc.Bacc(trn_type="TRN2", target_bir_lowering=False, debug=False)

    xT = nc.dram_tensor("xT", [C, T], BF16, kind="ExternalInput").ap()
    wq = nc.dram_tensor("wq", [C, WCOLS], BF16, kind="ExternalInput").ap()
    wk = nc.dram_tensor("wk", [C, WCOLS], BF16, kind="ExternalInput").ap()
    wv = nc.dram_tensor("wv", [C, WCOLS], BF16, kind="ExternalInput").ap()
    wp = nc.dram_tensor("wp", [WCOLS, C], BF16, kind="ExternalInput").ap()
    if use_bias:
        bq = nc.dram_tensor("bq", [WCOLS], F32, kind="ExternalInput").ap()
        bk = nc.dram_tensor("bk", [WCOLS], F32, kind="ExternalInput").ap()
        bv = nc.dram_tensor("bv", [WCOLS], F32, kind="ExternalInput").ap()
    out = nc.dram_tensor("out", [T, C], F32, kind="ExternalOutput").ap()

    with tile.TileContext(nc) as tc, ExitStack() as ctx:
        pool = ctx.enter_context(tc.tile_pool(name="main", bufs=1))
        xpool = ctx.enter_context(tc.tile_pool(name="xt", bufs=2))
        ptpool = ctx.enter_context(tc.tile_pool(name="pt", bufs=40))
        ynpool = ctx.enter_context(tc.tile_pool(name="yn", bufs=4))
        zpool = ctx.enter_context(tc.tile_pool(name="zr", bufs=4))
        opool = ctx.enter_context(tc.tile_pool(name="out", bufs=3))
        ps_mm = ctx.enter_context(tc.tile_pool(name="ps_mm", bufs=2, space="PSUM"))
        ps_s = ctx.enter_context(tc.tile_pool(name="ps_s", bufs=2, space="PSUM"))
        ps_y = ctx.enter_context(tc.tile_pool(name="ps_y", bufs=2, space="PSUM"))

        QT = [pool.tile([128, T], BF16, tag=f"qt{p}", name=f"qt{p}") for p in range(NPAIR)]
        KT = [pool.tile([128, T], BF16, tag=f"kt{p}", name=f"kt{p}") for p in range(NPAIR)]
        # V tiles head-major with a trailing ones column per head: [tok, h, 65]
        V = [pool.tile([128, HPC, DH + 1], BF16, tag=f"v{t}", name=f"v{t}") for t in range(KC)]
        for t in range(KC):
            nc.vector.memset(V[t][:, :, DH : DH + 1], 1.0)
        YT = [pool.tile([128, T], BF16, tag=f"yt{p}", name=f"yt{p}") for p in range(NPAIR)]

        # PE p-state warm-up: ~4us of dependency-free zero matmuls so the
        # tensor engine reaches full clock before the first real operands
        # arrive from HBM (ramp needs ~3us of continuous busy).
        ident = pool.tile([128, 128], BF16, tag="ident", name="ident")
        make_identity(nc, ident)
        zdummy = pool.tile([128, 512], BF16, tag="zdummy", name="zdummy")
        nc.vector.memset(zdummy, 0.0)
        for wi in range(8):
            wps = ps_y.tile([128, 512], F32, tag="y", name="warm")
            nc.tensor.matmul(
                wps, lhsT=zdummy[:, 0:128], rhs=zdummy, start=True, stop=True
            )

        wq_t = pool.tile([128, FC, WCOLS], BF16, tag="wq", name="wq_t")
        wk_t = pool.tile([128, FC, WCOLS], BF16, tag="wk", name="wk_t")
        wv_t = pool.tile([128, FC, WCOLS], BF16, tag="wv", name="wv_t")
        wp_t = pool.tile([128, NPAIR, C], BF16, tag="wp", name="wp_t")
        wq_sb = [wq_t[:, f, :] for f in range(FC)]
        wk_sb = [wk_t[:, f, :] for f in range(FC)]
        wv_sb = [wv_t[:, f, :] for f in range(FC)]
        wp_sb = [wp_t[:, p, :] for p in range(NPAIR)]

        def issue_xt(t4):
            tok = slice(t4 * 512, (t4 + 1) * 512)
            xt_t = xpool.tile([128, FC, 512], BF16, tag="x", name="x")
            nc.sync.dma_start(
                xt_t, xT.rearrange("(f p) t -> p f t", p=128)[:, :, tok]
            )
            return [xt_t[:, f, :] for f in range(FC)]

        # DMA issue order sets queue priority: slab-0 activations and the
        # first-needed weights land first, wp (only needed by proj) last.
        xt0_t = xpool.tile([128, FC, 512], BF16, tag="x", name="x")
        xr = xT.rearrange("(f p) t -> p f t", p=128)
        wvr = wv.rearrange("(f p) n -> p f n", p=128)
        for jj in range(4):
            nc.sync.dma_start(xt0_t[:, 2*jj:2*jj+2, :], xr[:, 2*jj:2*jj+2, 0:512])
            nc.sync.dma_start(wv_t[:, 2*jj:2*jj+2, :], wvr[:, 2*jj:2*jj+2, :])
        xt0 = [xt0_t[:, f, :] for f in range(FC)]
        wqr = wq.rearrange("(f p) n -> p f n", p=128)
        nc.sync.dma_start(wq_t[:, 0:4, :], wqr[:, 0:4, :])
        nc.sync.dma_start(wq_t[:, 4:8, :], wqr[:, 4:8, :])
        nc.sync.dma_start(wk_t, wk.rearrange("(f p) n -> p f n", p=128))
        nc.sync.dma_start(wp_t, wp.rearrange("(g p) n -> p g n", p=128))

        if use_bias:
            bq_sb = pool.tile([128, NPAIR], F32)
            bk_sb = pool.tile([128, NPAIR], F32)
            nc.sync.dma_start(bq_sb, bq.rearrange("(c p) -> p c", p=128))
            nc.sync.dma_start(bk_sb, bk.rearrange("(c p) -> p c", p=128))
            bv_sb = pool.tile([128, WCOLS], F32)
            bv_bcast = bass.AP(
                tensor=bv.tensor, offset=bv.offset, ap=[[0, 128], *bv.ap]
            )
            nc.sync.dma_start(bv_sb, bv_bcast)

        # ====== fully interleaved pipeline over 512-token slabs ======

        def emit_qkv_slab(t4, xt=None):
            tok = slice(t4 * 512, (t4 + 1) * 512)
            if xt is None:
                xt = issue_xt(t4)

            # Emission order inside a slab: pair-p Q^T/K^T first so
            # attention for pair p unblocks after 2 psum groups, V chunks
            # spread between (needed only by the trailing Y phase).
            def emit_v(tt):
                kci = t4 * 4 + tt
                ps = ps_mm.tile([128, 512], F32, tag="ps", name="ps")
                for f in range(FC):
                    nc.tensor.matmul(
                        ps,
                        lhsT=xt[f][:, tt * 128 : (tt + 1) * 128],
                        rhs=wv_sb[f],
                        start=(f == 0),
                        stop=(f == FC - 1),
                    )
                psv = ps.rearrange("p (h d) -> p h d", h=HPC)
                if use_bias:
                    nc.vector.tensor_add(
                        V[kci][:, :, 0:DH],
                        psv,
                        bv_sb.rearrange("p (h d) -> p h d", h=HPC),
                    )
                else:
                    nc.vector.tensor_copy(V[kci][:, :, 0:DH], psv)

            def emit_qk(wsb, dst, bias, p):
                ps = ps_mm.tile([128, 512], F32, tag="ps", name="ps")
                for f in range(FC):
                    nc.tensor.matmul(
                        ps,
                        lhsT=wsb[f][:, p * 128 : (p + 1) * 128],
                        rhs=xt[f],
                        start=(f == 0),
                        stop=(f == FC - 1),
                    )
                if use_bias:
                    bsb = bq_sb if bias == "bq" else bk_sb
                    nc.scalar.activation(
                        dst[p][:, tok], ps, AF.Copy, bias=bsb[:, p : p + 1]
                    )
                else:
                    nc.vector.tensor_copy(dst[p][:, tok], ps)

            for tt in range(4):
                emit_v(tt)
            for p in range(NPAIR):
                emit_qk(wq_sb, QT, "bq", p)
                emit_qk(wk_sb, KT, "bk", p)

        PTS = {}

        def emit_attention_s(p, q):
            nblk = 4 * q + 4
            # --- S + exp phase: P~^T tiles [k, h, q] for all k blocks ---
            pts = []
            for k in range(nblk):
                # diagonal offset: columns q < d of this block are
                # fully masked -> restrict all work to q >= d
                d = max(0, 128 * k - 512 * q)
                # S^T block [128 k, 512-d q], both heads row-tiled
                s = ps_s.tile([128, 1024], F32, tag="s", name="s")
                pt = ptpool.tile([128, 2, 512], BF16, tag="pt", name="pt")
                for h in (0, 1):
                    nc.tensor.matmul(
                        s[:, h * 512 + d : (h + 1) * 512],
                        lhsT=KT[p][h * 64 : (h + 1) * 64, k * 128 : (k + 1) * 128],
                        rhs=QT[p][h * 64 : (h + 1) * 64, q * 512 + d : (q + 1) * 512],
                        start=True,
                        stop=True,
                    )
                sv = s.rearrange("p (h q) -> p h q", h=2)
                nc.scalar.activation(
                    pt[:, :, d:512], sv[:, :, d:512], AF.Exp, scale=SCALE
                )
                if k >= 4 * q:
                    # triangular boundary band: zero where q_b < k
                    nc.gpsimd.affine_select(
                        out=pt[:, :, d : d + 128],
                        in_=pt[:, :, d : d + 128],
                        compare_op=ALU.is_ge,
                        fill=0.0,
                        base=0,
                        channel_multiplier=-1,
                        pattern=[[0, 2], [1, 128]],
                    )
                pts.append(pt)
            PTS[(p, q)] = pts

        def emit_attention_y(p, q):
            pts = PTS.pop((p, q))
            # --- Y phase: q-major accumulation, one q-subtile at a time.
            # The two heads' accumulation groups MUST live in different PSUM
            # banks (interleaved groups in one bank corrupt each other), so
            # each head gets its own ring slot.  The feature-major transpose
            # goes through the DMA xbar, not the PE/PSUM. ---
            for qq in range(4):
                qsub = 4 * q + qq  # global 128-token row block
                yh = [ps_y.tile([128, 65], F32, tag="y", name=f"yh{h}") for h in (0, 1)]
                for k in range(qsub + 1):
                    for h in (0, 1):
                        nc.tensor.matmul(
                            yh[h],
                            lhsT=pts[k][:, h, qq * 128 : (qq + 1) * 128],
                            rhs=V[k][:, 2 * p + h, :],
                            start=(k == 0),
                            stop=(k == qsub),
                        )
                zi = zpool.tile([128, 2], F32, tag="zi", name="zi")
                yn = ynpool.tile([128, 128], BF16, tag="yn", name="yn")
                for h in (0, 1):
                    nc.vector.reciprocal(zi[:, h : h + 1], yh[h][:, 64:65])
                    nc.vector.tensor_scalar_mul(
                        yn[:, h * 64 : (h + 1) * 64], yh[h][:, 0:64], zi[:, h : h + 1]
                    )
                if q == QC - 1:
                    # very last subtile: PE transpose via the y-ring (its
                    # slot naturally waits this subtile's norm) -- shorter
                    # critical chain than the DMA xbar for the tail
                    tr = ps_y.tile([128, 128], BF16, tag="y", name="tr")
                    nc.tensor.transpose(tr, yn, ident)
                    nc.vector.tensor_copy(
                        YT[p][:, qsub * 128 : (qsub + 1) * 128], tr
                    )
                else:
                    nc.sync.dma_start_transpose(
                        YT[p][:, qsub * 128 : (qsub + 1) * 128], yn
                    )

        def emit_proj(tt):
            # proj psum lives in the qkv ring but is emitted after all qkv,
            # so its slot-reuse never gates qkv; consumers are un-gated
            # copies.  One merged [128,1024] DMA per token block (each DMA
            # costs a fixed slot on the single hardware DGE queue).
            o = opool.tile([128, C], F32, tag="o", name="o")
            for n2 in range(2):
                nsl = slice(n2 * 512, (n2 + 1) * 512)
                ps = ps_mm.tile([128, 512], F32, tag="ps", name="psp")
                for p in range(NPAIR):
                    nc.tensor.matmul(
                        ps,
                        lhsT=YT[p][:, tt * 128 : (tt + 1) * 128],
                        rhs=wp_sb[p][:, nsl],
                        start=(p == 0),
                        stop=(p == NPAIR - 1),
                    )
                nc.vector.tensor_copy(o[:, nsl], ps)
                if tt >= 4 * QC - 2:
                    # drain the final token blocks in halves so the out DMA
                    # overlaps the second half's copy
                    nc.sync.dma_start(out[tt * 128 : (tt + 1) * 128, nsl], o[:, nsl])
            if tt < 4 * QC - 2:
                nc.sync.dma_start(out[tt * 128 : (tt + 1) * 128, :], o)

        # Priority order (emission = scheduler priority): attention(q) above
        # qkv(q+1), with all proj at the lowest priority so its PE work acts
        # as stall filler during the exp-bound late attention windows.
        # Priority shape: S/exp feeds the Activation engine (the long serial
        # chain) as early as possible; each pair's Y phase trails one pair
        # behind its S phase (act-free PE work = stall filler), qkv(q+1) and
        # proj(q) below the chunk's attention.
        emit_qkv_slab(0, xt0)
        for q in range(QC):
            emit_attention_s(0, q)
            for p in range(1, NPAIR):
                emit_attention_s(p, q)
                emit_attention_y(p - 1, q)
            emit_attention_y(NPAIR - 1, q)
            if q + 1 < QC:
                emit_qkv_slab(q + 1)
        for tt in range(4 * QC):
            emit_proj(tt)

    nc.compile()
    return nc


_PROGRAMS: dict = {}


def _get_program(use_bias: bool):
    if use_bias not in _PROGRAMS:
        _PROGRAMS[use_bias] = _build_program(use_bias)
    return _PROGRAMS[use_bias]


def _bf16(a):
    return np.ascontiguousarray(a.astype(ml_dtypes.bfloat16))


def kernel(x, W_qkv, b_qkv, W_proj, b_proj):
    x = np.asarray(x, dtype=np.float32)
    W_qkv = np.asarray(W_qkv, dtype=np.float32)
    b_qkv = np.asarray(b_qkv, dtype=np.float32)
    W_proj = np.asarray(W_proj, dtype=np.float32)
    b_proj = np.asarray(b_proj, dtype=np.float32)

    use_bias = bool(np.any(b_qkv != 0.0))
    nc = _get_program(use_bias)

    xTb = np.ascontiguousarray(x.transpose(0, 2, 1))  # [B, C, T] f32

    in_maps = []
    for c in range(NCORES):
        b, s = c // 2, c % 2
        m = {
            "xT": _bf16(xTb[b]),
            "wq": _bf16(W_qkv[:, s * WCOLS : (s + 1) * WCOLS]),
            "wk": _bf16(W_qkv[:, C + s * WCOLS : C + (s + 1) * WCOLS]),
            "wv": _bf16(W_qkv[:, 2 * C + s * WCOLS : 2 * C + (s + 1) * WCOLS]),
            "wp": _bf16(W_proj[s * WCOLS : (s + 1) * WCOLS, :]),
        }
        if use_bias:
            m["bq"] = np.ascontiguousarray(b_qkv[s * WCOLS : (s + 1) * WCOLS])
            m["bk"] = np.ascontiguousarray(b_qkv[C + s * WCOLS : C + (s + 1) * WCOLS])
            m["bv"] = np.ascontiguousarray(
                b_qkv[2 * C + s * WCOLS : 2 * C + (s + 1) * WCOLS]
            )
        in_maps.append(m)

    res = run_bass_kernel_spmd(nc, in_maps, list(range(NCORES))).results

    outp = np.empty((B, T, C), dtype=np.float32)
    for b in range(B):
        outp[b] = res[2 * b]["out"] + res[2 * b + 1]["out"]
    outp += b_proj
    return outp


def modeled_ns(use_bias: bool = False) -> float:
    """Single-core cost-model estimate of the kernel duration."""
    from concourse.timeline_sim import TimelineSim

    return TimelineSim(_build_program(use_bias)).simulate()


# revision 7
# speedup vs baseline: 1.0393x; 1.0024x over previous
"""Causal self-attention (B=4, T=2048, C=1024, 16 heads) on 8 Trainium2 cores.

Sharding: core c -> batch b = c//2 (4 data-parallel groups), head shard
s = c%2 (Megatron tensor-parallel: 8 of 16 heads, qkv column-sharded,
proj row-sharded).  Each core computes a partial projection output for
its batch; the host sums the two partials per batch (+ b_proj).

Pipeline design (evolved from a Y^T-oriented baseline, 365us -> 224us
modeled):
  * P~@V is q-major: lhsT = P~^T block [k,q-sub], rhs = V-hat [k, 65]
    so each matmul streams 65 columns instead of 512 (tensor-engine
    cost is purely the moving-operand free size).  The ones-column of
    V-hat lands the softmax denominator Z as a PSUM *column*, making
    normalization a per-partition reciprocal + tensor_scalar multiply.
  * The two heads' PV accumulation groups live in separate PSUM ring
    slots (= separate banks): interleaved accumulation groups sharing
    a bank corrupt each other.
  * The normalized [q, c] tile returns to feature-major via the DMA
    xbar (dma_start_transpose), keeping the PE and PSUM out of it.
  * Emission order = scheduler priority + pool-ring slot order: S/exp
    feeds the Activation engine (the serial softmax chain) as early as
    possible; each pair's Y phase trails one pair; qkv(q+1) sits below
    attention(q); all projection work is emitted last as pure stall
    filler.  proj shares the qkv psum ring but never precedes a qkv
    slab there, so ring reuse cannot gate qkv.
  * DMAs are merged (one per weight tensor / x-slab / out token-block,
    plus split first-arrivals for the prologue): each DMA costs a
    fixed slot on the single hardware DGE queue.
  * A short burst of zero-matmuls at t=0 ramps the PE p-state while
    the first DMAs land.
"""

import numpy as np
import ml_dtypes
from contextlib import ExitStack

import concourse.bass as bass
import concourse.tile as tile
from concourse.masks import make_identity
from concourse import mybir, bacc
from concourse.bass_utils import run_bass_kernel_spmd

F32 = mybir.dt.float32
BF16 = mybir.dt.bfloat16
AF = mybir.ActivationFunctionType
ALU = mybir.AluOpType

B, T, C = 4, 2048, 1024
NH, DH = 16, 64
SCALE = 1.0 / float(np.sqrt(DH))
NCORES = 8
HPC = 8              # heads per core
WCOLS = HPC * DH     # 512 qkv columns per core
NPAIR = HPC // 2     # head pairs (row/psum packing unit)
KC = T // 128        # 16 key-token chunks
QC = T // 512        # 4 query chunks
FC = C // 128        # 8 feature chunks


def _build_program(use_bias: bool):
    nc = bacc.Bacc(trn_type="TRN2", target_bir_lowering=False, debug=False)

    xT = nc.dram_tensor("xT", [C, T], BF16, kind="ExternalInput").ap()
    wq = nc.dram_tensor("wq", [C, WCOLS], BF16, kind="ExternalInput").ap()
    wk = nc.dram_tensor("wk", [C, WCOLS], BF16, kind="ExternalInput").ap()
    wv = nc.dram_tensor("wv", [C, WCOLS], BF16, kind="ExternalInput").ap()
    wp = nc.dram_tensor("wp", [WCOLS, C], BF16, kind="ExternalInput").ap()
    if use_bias:
        bq = nc.dram_tensor("bq", [WCOLS], F32, kind="ExternalInput").ap()
        bk = nc.dram_tensor("bk", [WCOLS], F32, kind="ExternalInput").ap()
        bv = nc.dram_tensor("bv", [WCOLS], F32, kind="ExternalInput").ap()
    # partial projection sums leave as bf16: host upcasts before summing the
    # two tensor-parallel partials (rounding adds ~0.2% of partial scale,
    # well inside the error budget; halves the output-drain DMA bytes)
    out = nc.dram_tensor("out", [T, C], BF16, kind="ExternalOutput").ap()

    with tile.TileContext(nc) as tc, ExitStack() as ctx:
        pool = ctx.enter_context(tc.tile_pool(name="main", bufs=1))
        xpool = ctx.enter_context(tc.tile_pool(name="xt", bufs=2))
        ptpool = ctx.enter_context(tc.tile_pool(name="pt", bufs=40))
        ynpool = ctx.enter_context(tc.tile_pool(name="yn", bufs=4))
        zpool = ctx.enter_context(tc.tile_pool(name="zr", bufs=4))
        opool = ctx.enter_context(tc.tile_pool(name="out", bufs=3))
        ps_mm = ctx.enter_context(tc.tile_pool(name="ps_mm", bufs=2, space="PSUM"))
        ps_s = ctx.enter_context(tc.tile_pool(name="ps_s", bufs=2, space="PSUM"))
        ps_y = ctx.enter_context(tc.tile_pool(name="ps_y", bufs=2, space="PSUM"))

        QT = [pool.tile([128, T], BF16, tag=f"qt{p}", name=f"qt{p}") for p in range(NPAIR)]
        KT = [pool.tile([128, T], BF16, tag=f"kt{p}", name=f"kt{p}") for p in range(NPAIR)]
        # V tiles head-major with a trailing ones column per head: [tok, h, 65]
        V = [pool.tile([128, HPC, DH + 1], BF16, tag=f"v{t}", name=f"v{t}") for t in range(KC)]
        for t in range(KC):
            nc.vector.memset(V[t][:, :, DH : DH + 1], 1.0)
        YT = [pool.tile([128, T], BF16, tag=f"yt{p}", name=f"yt{p}") for p in range(NPAIR)]

        # PE p-state warm-up: ~4us of dependency-free zero matmuls so the
        # tensor engine reaches full clock before the first real operands
        # arrive from HBM (ramp needs ~3us of continuous busy).
        ident = pool.tile([128, 128], BF16, tag="ident", name="ident")
        make_identity(nc, ident)
        zdummy = pool.tile([128, 512], BF16, tag="zdummy", name="zdummy")
        nc.vector.memset(zdummy, 0.0)
        for wi in range(8):
            wps = ps_y.tile([128, 512], F32, tag="y", name="warm")
            nc.tensor.matmul(
                wps, lhsT=zdummy[:, 0:128], rhs=zdummy, start=True, stop=True
            )

        wq_t = pool.tile([128, FC, WCOLS], BF16, tag="wq", name="wq_t")
        wk_t = pool.tile([128, FC, WCOLS], BF16, tag="wk", name="wk_t")
        wv_t = pool.tile([128, FC, WCOLS], BF16, tag="wv", name="wv_t")
        wp_t = pool.tile([128, NPAIR, C], BF16, tag="wp", name="wp_t")
        wq_sb = [wq_t[:, f, :] for f in range(FC)]
        wk_sb = [wk_t[:, f, :] for f in range(FC)]
        wv_sb = [wv_t[:, f, :] for f in range(FC)]
        wp_sb = [wp_t[:, p, :] for p in range(NPAIR)]

        def issue_xt(t4):
            tok = slice(t4 * 512, (t4 + 1) * 512)
            xt_t = xpool.tile([128, FC, 512], BF16, tag="x", name="x")
            nc.sync.dma_start(
                xt_t, xT.rearrange("(f p) t -> p f t", p=128)[:, :, tok]
            )
            return [xt_t[:, f, :] for f in range(FC)]

        # DMA issue order sets queue priority: slab-0 activations and the
        # first-needed weights land first, wp (only needed by proj) last.
        xt0_t = xpool.tile([128, FC, 512], BF16, tag="x", name="x")
        xr = xT.rearrange("(f p) t -> p f t", p=128)
        wvr = wv.rearrange("(f p) n -> p f n", p=128)
        for jj in range(4):
            nc.sync.dma_start(xt0_t[:, 2*jj:2*jj+2, :], xr[:, 2*jj:2*jj+2, 0:512])
            nc.sync.dma_start(wv_t[:, 2*jj:2*jj+2, :], wvr[:, 2*jj:2*jj+2, :])
        xt0 = [xt0_t[:, f, :] for f in range(FC)]
        wqr = wq.rearrange("(f p) n -> p f n", p=128)
        nc.sync.dma_start(wq_t[:, 0:4, :], wqr[:, 0:4, :])
        nc.sync.dma_start(wq_t[:, 4:8, :], wqr[:, 4:8, :])
        nc.sync.dma_start(wk_t, wk.rearrange("(f p) n -> p f n", p=128))
        nc.sync.dma_start(wp_t, wp.rearrange("(g p) n -> p g n", p=128))

        if use_bias:
            bq_sb = pool.tile([128, NPAIR], F32)
            bk_sb = pool.tile([128, NPAIR], F32)
            nc.sync.dma_start(bq_sb, bq.rearrange("(c p) -> p c", p=128))
            nc.sync.dma_start(bk_sb, bk.rearrange("(c p) -> p c", p=128))
            bv_sb = pool.tile([128, WCOLS], F32)
            bv_bcast = bass.AP(
                tensor=bv.tensor, offset=bv.offset, ap=[[0, 128], *bv.ap]
            )
            nc.sync.dma_start(bv_sb, bv_bcast)

        # ====== fully interleaved pipeline over 512-token slabs ======

        def emit_qkv_slab(t4, xt=None):
            tok = slice(t4 * 512, (t4 + 1) * 512)
            if xt is None:
                xt = issue_xt(t4)

            # Emission order inside a slab: pair-p Q^T/K^T first so
            # attention for pair p unblocks after 2 psum groups, V chunks
            # spread between (needed only by the trailing Y phase).
            def emit_v(tt):
                kci = t4 * 4 + tt
                ps = ps_mm.tile([128, 512], F32, tag="ps", name="ps")
                for f in range(FC):
                    nc.tensor.matmul(
                        ps,
                        lhsT=xt[f][:, tt * 128 : (tt + 1) * 128],
                        rhs=wv_sb[f],
                        start=(f == 0),
                        stop=(f == FC - 1),
                    )
                psv = ps.rearrange("p (h d) -> p h d", h=HPC)
                if use_bias:
                    nc.vector.tensor_add(
                        V[kci][:, :, 0:DH],
                        psv,
                        bv_sb.rearrange("p (h d) -> p h d", h=HPC),
                    )
                else:
                    nc.vector.tensor_copy(V[kci][:, :, 0:DH], psv)

            def emit_qk(wsb, dst, bias, p):
                ps = ps_mm.tile([128, 512], F32, tag="ps", name="ps")
                for f in range(FC):
                    nc.tensor.matmul(
                        ps,
                        lhsT=wsb[f][:, p * 128 : (p + 1) * 128],
                        rhs=xt[f],
                        start=(f == 0),
                        stop=(f == FC - 1),
                    )
                if use_bias:
                    bsb = bq_sb if bias == "bq" else bk_sb
                    nc.scalar.activation(
                        dst[p][:, tok], ps, AF.Copy, bias=bsb[:, p : p + 1]
                    )
                else:
                    nc.vector.tensor_copy(dst[p][:, tok], ps)

            for tt in range(4):
                emit_v(tt)
            for p in range(NPAIR):
                emit_qk(wq_sb, QT, "bq", p)
                emit_qk(wk_sb, KT, "bk", p)

        PTS = {}

        def emit_attention_s(p, q):
            nblk = 4 * q + 4
            # --- S + exp phase: P~^T tiles [k, h, q] for all k blocks ---
            pts = []
            for k in range(nblk):
                # diagonal offset: columns q < d of this block are
                # fully masked -> restrict all work to q >= d
                d = max(0, 128 * k - 512 * q)
                # S^T block [128 k, 512-d q], both heads row-tiled
                s = ps_s.tile([128, 1024], F32, tag="s", name="s")
                pt = ptpool.tile([128, 2, 512], BF16, tag="pt", name="pt")
                for h in (0, 1):
                    nc.tensor.matmul(
                        s[:, h * 512 + d : (h + 1) * 512],
                        lhsT=KT[p][h * 64 : (h + 1) * 64, k * 128 : (k + 1) * 128],
                        rhs=QT[p][h * 64 : (h + 1) * 64, q * 512 + d : (q + 1) * 512],
                        start=True,
                        stop=True,
                    )
                sv = s.rearrange("p (h q) -> p h q", h=2)
                nc.scalar.activation(
                    pt[:, :, d:512], sv[:, :, d:512], AF.Exp, scale=SCALE
                )
                if k >= 4 * q:
                    # triangular boundary band: zero where q_b < k
                    nc.gpsimd.affine_select(
                        out=pt[:, :, d : d + 128],
                        in_=pt[:, :, d : d + 128],
                        compare_op=ALU.is_ge,
                        fill=0.0,
                        base=0,
                        channel_multiplier=-1,
                        pattern=[[0, 2], [1, 128]],
                    )
                pts.append(pt)
            PTS[(p, q)] = pts

        def emit_attention_y(p, q):
            pts = PTS.pop((p, q))
            # --- Y phase: q-major accumulation, one q-subtile at a time.
            # The two heads' accumulation groups MUST live in different PSUM
            # banks (interleaved groups in one bank corrupt each other), so
            # each head gets its own ring slot.  The feature-major transpose
            # goes through the DMA xbar, not the PE/PSUM. ---
            for qq in range(4):
                qsub = 4 * q + qq  # global 128-token row block
                yh = [ps_y.tile([128, 65], F32, tag="y", name=f"yh{h}") for h in (0, 1)]
                for k in range(qsub + 1):
                    for h in (0, 1):
                        nc.tensor.matmul(
                            yh[h],
                            lhsT=pts[k][:, h, qq * 128 : (qq + 1) * 128],
                            rhs=V[k][:, 2 * p + h, :],
                            start=(k == 0),
                            stop=(k == qsub),
                        )
                zi = zpool.tile([128, 2], F32, tag="zi", name="zi")
                yn = ynpool.tile([128, 128], BF16, tag="yn", name="yn")
                for h in (0, 1):
                    nc.vector.reciprocal(zi[:, h : h + 1], yh[h][:, 64:65])
                    nc.vector.tensor_scalar_mul(
                        yn[:, h * 64 : (h + 1) * 64], yh[h][:, 0:64], zi[:, h : h + 1]
                    )
                if q == QC - 1:
                    # very last subtile: PE transpose via the y-ring (its
                    # slot naturally waits this subtile's norm) -- shorter
                    # critical chain than the DMA xbar for the tail
                    tr = ps_y.tile([128, 128], BF16, tag="y", name="tr")
                    nc.tensor.transpose(tr, yn, ident)
                    nc.vector.tensor_copy(
                        YT[p][:, qsub * 128 : (qsub + 1) * 128], tr
                    )
                else:
                    nc.sync.dma_start_transpose(
                        YT[p][:, qsub * 128 : (qsub + 1) * 128], yn
                    )

        def emit_proj(tt):
            # proj psum lives in the qkv ring but is emitted after all qkv,
            # so its slot-reuse never gates qkv; consumers are un-gated
            # copies.  One merged [128,1024] DMA per token block (each DMA
            # costs a fixed slot on the single hardware DGE queue).
            o = opool.tile([128, C], BF16, tag="o", name="o")
            for n2 in range(2):
                nsl = slice(n2 * 512, (n2 + 1) * 512)
                ps = ps_mm.tile([128, 512], F32, tag="ps", name="psp")
                for p in range(NPAIR):
                    nc.tensor.matmul(
                        ps,
                        lhsT=YT[p][:, tt * 128 : (tt + 1) * 128],
                        rhs=wp_sb[p][:, nsl],
                        start=(p == 0),
                        stop=(p == NPAIR - 1),
                    )
                if tt >= 4 * QC - 2:
                    # final token blocks: copies split across DVE and the
                    # (idle) Activation engine, DMA per half to overlap the
                    # drain chain
                    if n2 == 0:
                        nc.vector.tensor_copy(o[:, nsl], ps)
                    else:
                        nc.scalar.activation(o[:, nsl], ps, AF.Copy)
                    nc.sync.dma_start(out[tt * 128 : (tt + 1) * 128, nsl], o[:, nsl])
                else:
                    nc.vector.tensor_copy(o[:, nsl], ps)
            if tt < 4 * QC - 2:
                nc.sync.dma_start(out[tt * 128 : (tt + 1) * 128, :], o)

        # Priority order (emission = scheduler priority): attention(q) above
        # qkv(q+1), with all proj at the lowest priority so its PE work acts
        # as stall filler during the exp-bound late attention windows.
        # Priority shape: S/exp feeds the Activation engine (the long serial
        # chain) as early as possible; each pair's Y phase trails one pair
        # behind its S phase (act-free PE work = stall filler), qkv(q+1) and
        # proj(q) below the chunk's attention.
        emit_qkv_slab(0, xt0)
        for q in range(QC):
            emit_attention_s(0, q)
            for p in range(1, NPAIR):
                emit_attention_s(p, q)
                emit_attention_y(p - 1, q)
            emit_attention_y(NPAIR - 1, q)
            if q + 1 < QC:
                emit_qkv_slab(q + 1)
        for tt in range(4 * QC):
            emit_proj(tt)

    nc.compile()
    return nc


_PROGRAMS: dict = {}


def _get_program(use_bias: bool):
    if use_bias not in _PROGRAMS:
        _PROGRAMS[use_bias] = _build_program(use_bias)
    return _PROGRAMS[use_bias]


def _bf16(a):
    return np.ascontiguousarray(a.astype(ml_dtypes.bfloat16))


def kernel(x, W_qkv, b_qkv, W_proj, b_proj):
    x = np.asarray(x, dtype=np.float32)
    W_qkv = np.asarray(W_qkv, dtype=np.float32)
    b_qkv = np.asarray(b_qkv, dtype=np.float32)
    W_proj = np.asarray(W_proj, dtype=np.float32)
    b_proj = np.asarray(b_proj, dtype=np.float32)

    use_bias = bool(np.any(b_qkv != 0.0))
    nc = _get_program(use_bias)

    xTb = np.ascontiguousarray(x.transpose(0, 2, 1))  # [B, C, T] f32

    in_maps = []
    for c in range(NCORES):
        b, s = c // 2, c % 2
        m = {
            "xT": _bf16(xTb[b]),
            "wq": _bf16(W_qkv[:, s * WCOLS : (s + 1) * WCOLS]),
            "wk": _bf16(W_qkv[:, C + s * WCOLS : C + (s + 1) * WCOLS]),
            "wv": _bf16(W_qkv[:, 2 * C + s * WCOLS : 2 * C + (s + 1) * WCOLS]),
            "wp": _bf16(W_proj[s * WCOLS : (s + 1) * WCOLS, :]),
        }
        if use_bias:
            m["bq"] = np.ascontiguousarray(b_qkv[s * WCOLS : (s + 1) * WCOLS])
            m["bk"] = np.ascontiguousarray(b_qkv[C + s * WCOLS : C + (s + 1) * WCOLS])
            m["bv"] = np.ascontiguousarray(
                b_qkv[2 * C + s * WCOLS : 2 * C + (s + 1) * WCOLS]
            )
        in_maps.append(m)

    res = run_bass_kernel_spmd(nc, in_maps, list(range(NCORES))).results

    outp = np.empty((B, T, C), dtype=np.float32)
    for b in range(B):
        outp[b] = res[2 * b]["out"].astype(np.float32) + res[2 * b + 1][
            "out"
        ].astype(np.float32)
    outp += b_proj
    return outp


def modeled_ns(use_bias: bool = False) -> float:
    """Single-core cost-model estimate of the kernel duration."""
    from concourse.timeline_sim import TimelineSim

    return TimelineSim(_build_program(use_bias)).simulate()
